# revision 36
# baseline (speedup 1.0000x reference)
# BinsCombinerLayer Trainium2 kernel.
#
#   out[b] = (1/NUM_BINS) * sum_{n,s} inputs[b,n,s] * centroids[n,s]
#
# Pure data parallel over 8 NeuronCores: each core takes B/8 = 4096 examples.
# The dot product runs on the PE array in bf16 (the 2e-2 tolerance leaves
# orders of magnitude of slack), which halves the HBM traffic vs f32 --
# the binding roofline for this kernel -- and frees the DVE entirely.
#
# Host-side prep per core: x slice [4096, 2048] f32 is cast to bf16 and
# transposed to xT [2048, 4096] (d-major) so the PE can contract over the
# partition axis: for each 128-row d-chunk k, matmul(psum[1, N], lhsT =
# cbT[:, k] [128, 1], rhs = xT_k [128, N]) accumulates the per-example
# partial dots over the 16 chunks in PSUM.  Centroids are pre-scaled by
# 1/NUM_BINS and transposed to [128, 16] on host (tiny).
import numpy as np

import concourse.bacc as bacc
import concourse.mybir as mybir
import concourse.tile as tile
from concourse.bass_utils import run_bass_kernel_spmd

N_CORES = 8
B, NUM_BINS, BIN_SIZE = 32768, 16, 128
D = NUM_BINS * BIN_SIZE      # 2048 f32 per example
P = 128                      # SBUF partitions
BC = B // N_CORES            # 4096 examples per core
K = D // P                   # 16 d-chunks of 128
F32 = mybir.dt.float32
BF16 = mybir.dt.bfloat16
U8 = mybir.dt.uint8

_CACHED = None


def _build_program(repeat=1, qw=1024, nblk=512, bufs=12, dual_q=False,
                   out_q="sync", tilemajor=True, mm_width=None):
    """Per pass: 4 quarters of qw examples; per quarter, 16 chunk DMAs
    (one per 128-row d-chunk) feed qw/nblk PSUM accumulation groups.
    Matmuls run k-outer within the quarter so chunk k's matmuls fire as its
    DMA lands and the post-last-DMA tail is just the final k's matmuls plus
    the PSUM drains; drain copies alternate ACT/DVE."""
    nc = bacc.Bacc("TRN2", target_bir_lowering=False, debug=False)
    nq = BC // qw
    if tilemajor:
        # tile-major DRAM layout: each (quarter, chunk) DMA reads one fully
        # contiguous 128*qw*2B extent (better HBM row locality)
        x = nc.dram_tensor("x", [nq * K * P, qw], BF16, kind="ExternalInput").ap()
    else:
        x = nc.dram_tensor("x", [D, BC], BF16, kind="ExternalInput").ap()
    cb = nc.dram_tensor("cb", [P, K], BF16, kind="ExternalInput").ap()
    out = nc.dram_tensor("out", [1, BC], F32, kind="ExternalOutput").ap()
    NB = qw // nblk
    with tile.TileContext(nc) as tc:
        with (
            tc.tile_pool(name="xin", bufs=bufs) as xpool,
            tc.tile_pool(name="misc", bufs=1) as misc,
            tc.tile_pool(
                name="ps", bufs=min(8, max(4, 2 * NB)), space="PSUM"
            ) as pspool,
        ):
            cbt = misc.tile([P, K], BF16)
            # scalar (ACT) HWDGE queue: runs parallel to the x stream on sync
            nc.scalar.dma_start(out=cbt[:], in_=cb[:])
            collect = misc.tile([1, BC], F32)

            for _ in range(repeat):
                for q in range(nq):
                    xts = []
                    for k in range(K):
                        xt = xpool.tile([P, qw], BF16, tag="xt")
                        eng = nc.scalar if (dual_q and k % 2) else nc.sync
                        if tilemajor:
                            r = (q * K + k) * P
                            src = x[r : r + P, :]
                        else:
                            src = x[k * P : (k + 1) * P, q * qw : (q + 1) * qw]
                        eng.dma_start(out=xt[:], in_=src)
                        xts.append(xt)
                    pss = [
                        pspool.tile([1, nblk], F32, tag="ps", name=f"ps_{b}")
                        for b in range(NB)
                    ]
                    mw = mm_width or nblk
                    for k in range(K):
                        for blk in range(NB):
                            lo = blk * nblk
                            nc.tensor.matmul(
                                pss[blk][:, :mw],
                                cbt[:, k : k + 1],
                                xts[k][:, lo : lo + mw],
                                start=(k == 0),
                                stop=(k == K - 1),
                            )
                    for blk in range(NB):
                        seg = collect[
                            :, q * qw + blk * nblk : q * qw + (blk + 1) * nblk
                        ]
                        if blk % 2:
                            nc.vector.tensor_copy(seg, pss[blk][:])
                        else:
                            nc.scalar.copy(seg, pss[blk][:])

                getattr(nc, out_q).dma_start(out=out[:], in_=collect[:])

    nc.compile()
    return nc


def _get_program():
    global _CACHED
    if _CACHED is None:
        _CACHED = _build_program_v6()
    return _CACHED


# ---- v6: PE + DVE split (the active path) ---------------------------------- -----------------------------------------------------
# The DMA stream sustains ~740 GB/s/core but the PE alone caps the pass at
# ~27.5 us (1 bf16 column/cycle).  Offload the last PE_E..BC examples to the
# (otherwise idle) DVE as 1x scalar_tensor_tensor dots in example-major
# layout, balancing PE ~21 us / DVE ~18 us under the ~23 us DMA floor.
PE_E = 3072            # examples on the PE (3 tile-major quarters)
DVE_E = BC - PE_E      # 1024 examples on the DVE
TD = DVE_E // P        # 8 example-slots per partition


def _build_program_v6(repeat=1, qw=1024, nblk=512, bufs=12, dve_u8=True):
    nc = bacc.Bacc("TRN2", target_bir_lowering=False, debug=False)
    nq = PE_E // qw
    edt = U8 if dve_u8 else BF16
    x = nc.dram_tensor("x", [nq * K * P, qw], BF16, kind="ExternalInput").ap()
    xe = nc.dram_tensor("xe", [(TD // 2) * P, 2 * D], edt, kind="ExternalInput").ap()
    cb = nc.dram_tensor("cb", [P, K], BF16, kind="ExternalInput").ap()
    cbb = nc.dram_tensor("cbb", [P, D], BF16, kind="ExternalInput").ap()
    out = nc.dram_tensor("out", [1, PE_E], F32, kind="ExternalOutput").ap()
    out2 = nc.dram_tensor("out2", [P, TD], F32, kind="ExternalOutput").ap()
    NB = qw // nblk
    with tile.TileContext(nc) as tc:
        with (
            tc.tile_pool(name="xin", bufs=bufs) as xpool,
            tc.tile_pool(name="xein", bufs=4) as xepool,
            tc.tile_pool(name="misc", bufs=1) as misc,
            tc.tile_pool(name="ps", bufs=6, space="PSUM") as pspool,
        ):
            cbt = misc.tile([P, K], BF16)
            nc.scalar.dma_start(out=cbt[:], in_=cb[:])
            cbbt = misc.tile([P, D], BF16)
            nc.scalar.dma_start(out=cbbt[:], in_=cbb[:])
            collect = misc.tile([1, PE_E], F32)
            colle = misc.tile([P, TD], F32)
            # STT elementwise result goes to an SBUF scratch tile: a PSUM
            # dummy would contend with the PE's concurrent PSUM accumulation
            scratch = misc.tile([P, D], BF16)

            for _ in range(repeat):
                # DVE stream: TD//2 example-pair tiles, extent-contiguous.
                # Only the first two go ahead of the PE's quarter-0 tiles
                # (short ramp); the rest slot in after quarter 0 -- the DVE
                # has ~4 us of slack vs the PE so the later arrival hides.
                xets = {}

                def dma_xe(j):
                    xet = xepool.tile(
                        [P, 2 * D], edt, tag="xe", name=f"xe_{j}"
                    )
                    nc.sync.dma_start(
                        out=xet[:], in_=xe[j * P : (j + 1) * P, :]
                    )
                    xets[j] = xet

                def stt_pair(j):
                    for h in range(2):
                        t = 2 * j + h
                        nc.vector.scalar_tensor_tensor(
                            out=scratch[:],
                            in0=xets[j][:, h * D : (h + 1) * D],
                            scalar=1.0,
                            in1=cbbt[:],
                            op0=mybir.AluOpType.mult,
                            op1=mybir.AluOpType.mult,
                            accum_out=colle[:, t : t + 1],
                        )

                dma_xe(0)
                dma_xe(1)
                # PE stream: tile-major quarters
                for q in range(nq):
                    xts = []
                    for k in range(K):
                        xt = xpool.tile([P, qw], BF16, tag="xt")
                        r = (q * K + k) * P
                        nc.sync.dma_start(out=xt[:], in_=x[r : r + P, :])
                        xts.append(xt)
                    if q == 0:
                        stt_pair(0)
                        stt_pair(1)
                        for j in range(2, TD // 2):
                            dma_xe(j)
                        for j in range(2, TD // 2):
                            stt_pair(j)
                        # out2 only needs the STT accums: overlap its DMA
                        # with the remaining PE quarters, on the otherwise
                        # idle SWDGE queue so no sequencer stalls
                        nc.gpsimd.dma_start(out=out2[:], in_=colle[:])
                    pss = [
                        pspool.tile([1, nblk], F32, tag="ps", name=f"ps_{b}")
                        for b in range(NB)
                    ]
                    for k in range(K):
                        for blk in range(NB):
                            lo = blk * nblk
                            nc.tensor.matmul(
                                pss[blk][:],
                                cbt[:, k : k + 1],
                                xts[k][:, lo : lo + nblk],
                                start=(k == 0),
                                stop=(k == K - 1),
                            )
                    for blk in range(NB):
                        seg = collect[
                            :, q * qw + blk * nblk : q * qw + (blk + 1) * nblk
                        ]
                        if blk % 2:
                            nc.vector.tensor_copy(seg, pss[blk][:])
                        else:
                            nc.scalar.copy(seg, pss[blk][:])

                nc.sync.dma_start(out=out[:], in_=collect[:])

    nc.compile()
    return nc


def _prep_inputs_v6(inputs, centroids, qw=1024, dve_u8=True):
    import ml_dtypes

    bf16 = ml_dtypes.bfloat16
    x = np.asarray(inputs, dtype=np.float32).reshape(N_CORES, BC, D)
    xbf = x.astype(bf16)
    nq = PE_E // qw
    # PE part: tile-major d-major layout of the first PE_E examples
    xT = np.ascontiguousarray(
        xbf[:, :PE_E, :].transpose(0, 2, 1)
    )  # [cores, D, PE_E]
    xT = np.ascontiguousarray(
        xT.reshape(N_CORES, K, P, nq, qw).transpose(0, 3, 1, 2, 4)
    ).reshape(N_CORES, nq * K * P, qw)
    # DVE part: example-major, b = PE_E + TD*p + t, blocked into
    # extent-contiguous [P, 2*D] pair-tiles.  Shipped as uint8 with a
    # per-tensor scale folded into the replicated centroid table (the DVE
    # STT runs at 1x for any input dtype, so narrower bytes are free).
    c = np.asarray(centroids, dtype=np.float32).reshape(D) / NUM_BINS
    if dve_u8:
        xd_f = x[:, PE_E:, :]
        qx = float(xd_f.max()) / 255.0
        xd = np.round(xd_f / qx).astype(np.uint8).reshape(N_CORES, P, TD, D)
        cbb_row = (c * qx).astype(bf16)
    else:
        xd = xbf[:, PE_E:, :].reshape(N_CORES, P, TD, D)
        cbb_row = c.astype(bf16)
    xe = np.ascontiguousarray(
        xd.reshape(N_CORES, P, TD // 2, 2 * D).transpose(0, 2, 1, 3)
    ).reshape(N_CORES, (TD // 2) * P, 2 * D)
    cbT = np.ascontiguousarray(c.astype(bf16).reshape(K, P).T)
    cbb = np.ascontiguousarray(np.broadcast_to(cbb_row, (P, D)))
    return xT, xe, cbT, cbb


def _prep_inputs(inputs, centroids, qw=1024, tilemajor=True):
    import ml_dtypes

    bf16 = ml_dtypes.bfloat16
    x = np.asarray(inputs, dtype=np.float32).reshape(N_CORES, BC, D)
    # cast + transpose to per-core [D, BC] bf16 (d-major, examples contiguous)
    xT = np.ascontiguousarray(x.transpose(0, 2, 1)).astype(bf16)
    if tilemajor:
        nq = BC // qw
        # [cores, D, BC] -> [cores, nq*K*P, qw] with (q, k) tiles contiguous
        xT = np.ascontiguousarray(
            xT.reshape(N_CORES, K, P, nq, qw).transpose(0, 3, 1, 2, 4)
        ).reshape(N_CORES, nq * K * P, qw)
    c = np.asarray(centroids, dtype=np.float32).reshape(D) / NUM_BINS
    # cbT[p, k] = c[k*128 + p]
    cbT = np.ascontiguousarray(c.astype(bf16).reshape(K, P).T)
    return xT, cbT


def run(inputs, centroids, **spmd_kwargs):
    """Run the kernel; returns (full_output, BassKernelResults)."""
    nc = _get_program()
    xT, xe, cbT, cbb = _prep_inputs_v6(inputs, centroids)
    in_maps = [
        {"x": xT[i], "xe": xe[i], "cb": cbT, "cbb": cbb}
        for i in range(N_CORES)
    ]
    try:
        res = run_bass_kernel_spmd(
            nc, in_maps, list(range(N_CORES)), **spmd_kwargs
        )
    except Exception:
        # transient NRT_EXEC_UNIT_UNRECOVERABLE wedges recover on retry
        res = run_bass_kernel_spmd(
            nc, in_maps, list(range(N_CORES)), **spmd_kwargs
        )
    full = np.concatenate(
        [
            np.concatenate(
                [r["out"].reshape(PE_E), r["out2"].reshape(DVE_E)]
            )
            for r in res.results
        ]
    )
    return full.astype(np.float32, copy=False), res


def kernel(inputs, centroids):
    full, _ = run(inputs, centroids)
    return full

# ---- v8: rebalanced PE 2944 / DVE 1152 (u8) --------------------------------
# The PE (~20.7 us) near-gates v7 while the DVE (~18.3 us) has slack; moving
# one 128-example slot to the u8 DVE path shaves PE and DMA bytes.  Quarter 2
# is 896 wide: its DRAM rows stay 1024 wide (padded, never read) so the x
# tensor keeps one row shape; same trick for the DVE's odd 9th slot.
PE_E8 = 2944
DVE_E8 = BC - PE_E8   # 1152
TD8 = DVE_E8 // P     # 9 slots


def _build_program_v8(repeat=1, bufs=12):
    nc = bacc.Bacc("TRN2", target_bir_lowering=False, debug=False)
    QWS = [896, 1024, 1024]
    x = nc.dram_tensor("x", [len(QWS) * K * P, 1024], BF16, kind="ExternalInput").ap()
    xe = nc.dram_tensor("xe", [5 * P, 2 * D], U8, kind="ExternalInput").ap()
    cb = nc.dram_tensor("cb", [P, K], BF16, kind="ExternalInput").ap()
    cbb = nc.dram_tensor("cbb", [P, D], BF16, kind="ExternalInput").ap()
    out = nc.dram_tensor("out", [1, PE_E8], F32, kind="ExternalOutput").ap()
    out2 = nc.dram_tensor("out2", [P, TD8], F32, kind="ExternalOutput").ap()
    with tile.TileContext(nc) as tc:
        with (
            tc.tile_pool(name="xin", bufs=bufs) as xpool,
            tc.tile_pool(name="xin9", bufs=4) as xpool9,
            tc.tile_pool(name="xein", bufs=5) as xepool,
            tc.tile_pool(name="misc", bufs=1) as misc,
            tc.tile_pool(name="ps5", bufs=5, space="PSUM") as pspool512,
            tc.tile_pool(name="ps3", bufs=2, space="PSUM") as pspool384,
        ):
            cbt = misc.tile([P, K], BF16)
            nc.scalar.dma_start(out=cbt[:], in_=cb[:])
            cbbt = misc.tile([P, D], BF16)
            nc.scalar.dma_start(out=cbbt[:], in_=cbb[:])
            collect = misc.tile([1, PE_E8], F32)
            colle = misc.tile([P, TD8], F32)
            scratch = misc.tile([P, D], BF16)

            for _ in range(repeat):
                xets = {}

                def dma_xe(j):
                    if j < 4:
                        xet = xepool.tile([P, 2 * D], U8, tag="xe", name=f"xe_{j}")
                        nc.sync.dma_start(out=xet[:], in_=xe[j * P : (j + 1) * P, :])
                    else:
                        xet = xepool.tile([P, D], U8, tag="xe1", name="xe_4")
                        nc.sync.dma_start(out=xet[:], in_=xe[4 * P : 5 * P, 0:D])
                    xets[j] = xet

                def stt(j, h):
                    t = 2 * j + h
                    nc.vector.scalar_tensor_tensor(
                        out=scratch[:],
                        in0=xets[j][:, h * D : (h + 1) * D],
                        scalar=1.0,
                        in1=cbbt[:],
                        op0=mybir.AluOpType.mult,
                        op1=mybir.AluOpType.mult,
                        accum_out=colle[:, t : t + 1],
                    )

                dma_xe(0)
                dma_xe(1)
                off = 0
                for q, qw in enumerate(QWS):
                    xts = []
                    for k in range(K):
                        r = (q * K + k) * P
                        if qw == 1024:
                            xt = xpool.tile([P, qw], BF16, tag="xt")
                            nc.sync.dma_start(out=xt[:], in_=x[r : r + P, :])
                        else:
                            xt = xpool9.tile([P, qw], BF16, tag="xt9")
                            nc.sync.dma_start(out=xt[:], in_=x[r : r + P, 0:qw])
                        xts.append(xt)
                    if q == 0:
                        stt(0, 0); stt(0, 1); stt(1, 0); stt(1, 1)
                        for j in (2, 3, 4):
                            dma_xe(j)
                        stt(2, 0); stt(2, 1); stt(3, 0); stt(3, 1); stt(4, 0)
                        nc.gpsimd.dma_start(out=out2[:], in_=colle[:])
                    blks = [512] * (qw // 512) + ([qw % 512] if qw % 512 else [])
                    pss = [
                        (pspool512 if w == 512 else pspool384).tile(
                            [1, w], F32, tag=f"ps{w}", name=f"ps_{q}_{b}"
                        )
                        for b, w in enumerate(blks)
                    ]
                    for k in range(K):
                        lo = 0
                        for b, w in enumerate(blks):
                            nc.tensor.matmul(
                                pss[b][:],
                                cbt[:, k : k + 1],
                                xts[k][:, lo : lo + w],
                                start=(k == 0),
                                stop=(k == K - 1),
                            )
                            lo += w
                    lo = 0
                    for b, w in enumerate(blks):
                        seg = collect[:, off + lo : off + lo + w]
                        if b % 2:
                            nc.vector.tensor_copy(seg, pss[b][:])
                        else:
                            nc.scalar.copy(seg, pss[b][:])
                        lo += w
                    off += qw

                nc.sync.dma_start(out=out[:], in_=collect[:])

    nc.compile()
    return nc


def _prep_inputs_v8(inputs, centroids):
    import ml_dtypes

    bf16 = ml_dtypes.bfloat16
    x = np.asarray(inputs, dtype=np.float32).reshape(N_CORES, BC, D)
    xbf = x.astype(bf16)
    # PE part: quarters [1024, 1024, 896]; rows padded to 1024 wide
    xT = np.ascontiguousarray(xbf[:, :PE_E8, :].transpose(0, 2, 1))  # [c, D, 2944]
    rows = np.zeros((N_CORES, 3 * K * P, 1024), bf16)
    for q, (lo, w) in enumerate([(0, 896), (896, 1024), (1920, 1024)]):
        blk = xT[:, :, lo : lo + w].reshape(N_CORES, K, P, w)
        rows[:, q * K * P : (q + 1) * K * P, :w] = blk.reshape(N_CORES, K * P, w)
    # DVE part: b = PE_E8 + 9*p + t; 4 pair-tiles + padded single tile
    qx = float(x[:, PE_E8:, :].max()) / 255.0
    xd = np.round(x[:, PE_E8:, :] / qx).astype(np.uint8).reshape(N_CORES, P, TD8, D)
    xe = np.zeros((N_CORES, 5 * P, 2 * D), np.uint8)
    pairs = xd[:, :, :8, :].reshape(N_CORES, P, 4, 2 * D).transpose(0, 2, 1, 3)
    xe[:, : 4 * P, :] = pairs.reshape(N_CORES, 4 * P, 2 * D)
    xe[:, 4 * P :, :D] = xd[:, :, 8, :]
    c = np.asarray(centroids, dtype=np.float32).reshape(D) / NUM_BINS
    cbT = np.ascontiguousarray(c.astype(bf16).reshape(K, P).T)
    cbb = np.ascontiguousarray(np.broadcast_to((c * qx).astype(bf16), (P, D)))
    return rows, xe, cbT, cbb
